# revision 1
# baseline (speedup 1.0000x reference)
"""Bahdanau-attention scores kernel for Trainium2 (8 NeuronCores, SPMD).

Computation (per batch row b):
    pre[s, k] = hidden[b] @ Wh + enc[b, s] @ We + b_attn       (S=1024, E=K=1024)
    scores[s] = tanh(pre[s, :]) @ v
    out[b]    = softmax(where(mask[b]==0, -1e10, scores))      over s

Sharding: data-parallel over batch B=64 -> 8 batches per core; weights
replicated. No collectives.

Per-core structure (fp8 DoubleRow main matmul, bf16 elsewhere):
  - We is loaded row-pair-packed and scaled x64 into E4M3:
    w8[p, j, k] = 64 * We[et*256 + 2p + j, k]  (j in {0,1})
  - enc[b] is cast f32->fp8 by DRAM->DRAM SWDGE DMAs into a bounce buffer,
    then one DRAM->SBUF xbar transpose per batch moves the fp8 pairs as
    uint16: encT8[p, et', s] as u16 holds
    (enc[s, et'*256+2p], enc[s, et'*256+2p+1]) -- exactly the DoubleRow
    rhs pairing, so each 256-deep contraction needs one MM:
      pre[k, s] = sum_et' lhsT(w8) @ rhs(encT8)   [4 MMs per (kt, sb)]
  - ScalarE: tanh(psum/64 + (hidden@Wh + b_attn)[k]) -> SBUF bf16 (scale
    fuses the x64 We quantization scale)
  - v-dot: PE matmul lhsT=v[128,1] bf16, rhs=tanh half, accumulated over
    k-tiles into PSUM scores [1, 512]; DVE copies to a flat partition-0 row
  - SWDGE gathers flat scores into [BL, S]; masked softmax on DVE/ACT.

Sync note: this walrus build encodes at most ONE semaphore wait per
instruction; _split_multi_waits() rewrites Tile's multi-wait instructions
into NoOp(wait) chains on the same engine.
"""

import sys

if "/opt/trn_rl_repo" not in sys.path:
    sys.path.insert(0, "/opt/trn_rl_repo")

from contextlib import ExitStack

import numpy as np

B, S, E, K = 64, 1024, 1024, 1024  # E = 2*ENC_HID, K = DEC_HID
NCORES = 8
BL = B // NCORES  # batches per core
NEG = -1e10
WSCALE = 64.0     # We quantization scale into E4M3 range

_CACHE = {}


def _build_bass(strip=True):
    from concourse import bass, mybir, tile

    f32 = mybir.dt.float32
    bf16 = mybir.dt.bfloat16
    f8 = mybir.dt.float8e4
    u16 = mybir.dt.uint16
    i32 = mybir.dt.int32
    Tanh = mybir.ActivationFunctionType.Tanh
    Exp = mybir.ActivationFunctionType.Exp
    Alu = mybir.AluOpType
    Ax = mybir.AxisListType
    DR = mybir.MatmulPerfMode.DoubleRow

    nc = bass.Bass()

    enc_d = nc.declare_dram_parameter("encoder_outputs", [BL, S, E], f32, isOutput=False)
    enc8_d = nc.dram_tensor("enc8", [BL, S, E], mybir.dt.float8e4)
    mask_d = nc.declare_dram_parameter("mask", [BL, S], i32, isOutput=False)
    w_d = nc.declare_dram_parameter("W_attn", [2 * K, K], f32, isOutput=False)
    hT_d = nc.declare_dram_parameter("hiddenT", [K, BL], f32, isOutput=False)
    b_d = nc.declare_dram_parameter("b_attn", [K], f32, isOutput=False)
    vpt_d = nc.declare_dram_parameter("v_pt", [128, 8], f32, isOutput=False)
    out_d = nc.declare_dram_parameter("out", [BL, S], f32, isOutput=True)

    ET2 = E // 256  # 4 DoubleRow e-tiles (256-deep contraction each)
    KT = K // 128   # 8 k-tiles
    ST = S // 128   # 8 s-tiles
    NB = 512        # matmul free-dim block
    SB = S // NB    # 2 s-blocks

    with tile.TileContext(nc) as tc, ExitStack() as ctx:
        const = ctx.enter_context(tc.tile_pool(name="const", bufs=1))
        tp_pool = ctx.enter_context(tc.tile_pool(name="encT", bufs=3))
        tanh_pool = ctx.enter_context(tc.tile_pool(name="tanh", bufs=4))
        pre_ps = ctx.enter_context(tc.tile_pool(name="pre_ps", bufs=3, space="PSUM"))
        sc_ps = ctx.enter_context(tc.tile_pool(name="sc_ps", bufs=1, space="PSUM"))
        fin = ctx.enter_context(tc.tile_pool(name="fin", bufs=1))

        def stage_tp(b):
            """One DRAM->SBUF u16-pair xbar transpose of the fp8 bounce:
            encT8 u16[p, et', s] = fp8 pair
            (enc[b, s, et'*256+2p], enc[b, s, et'*256+2p+1])."""
            encT8 = tp_pool.tile([128, ET2, S], u16, tag="encT", name="encT8")
            nc.sync.dma_start(
                encT8[:], enc8_d[b].bitcast(u16), transpose=True)
            return encT8

        def stage_cast(b):
            for st in range(ST):
                nc.gpsimd.dma_start(
                    enc8_d[b, st * 128:(st + 1) * 128, :],
                    enc_d[b, st * 128:(st + 1) * 128, :])

        # ---- Wh (bf16), hiddenT, b_attn, v ----
        wh_bf = const.tile([128, KT, K], bf16)
        hT_bf = const.tile([128, KT, BL], bf16)
        b_attn_bf = const.tile([1, K], bf16)
        v_bf = const.tile([128, 8], bf16)
        with tc.tile_pool(name="stage", bufs=1) as stage:
            for t in range(KT):
                wst = stage.tile([128, K], f32, tag="wst", bufs=4)
                nc.sync.dma_start(wst[:], w_d[t * 128:(t + 1) * 128, :])
                nc.vector.tensor_copy(wh_bf[:, t, :], wst[:])

            hst = stage.tile([128, KT, BL], f32, tag="hst")
            nc.sync.dma_start(hst[:], hT_d.rearrange("(dt p) b -> p dt b", p=128))
            nc.vector.tensor_copy(hT_bf[:], hst[:])

            bst = stage.tile([1, K], f32, tag="bst")
            nc.sync.dma_start(bst[:], b_d[:])
            nc.vector.tensor_copy(b_attn_bf[:], bst[:])

            vst = stage.tile([128, 8], f32, tag="vst")
            nc.sync.dma_start(vst[:], vpt_d[:])
            nc.vector.tensor_copy(v_bf[:], vst[:])

        # ---- We: f32 row-pair loads -> DVE x64 scale-cast to E4M3 ----
        # w8[p, et', j, k] = WSCALE * We[et'*256 + 2p + j, k]
        w8 = const.tile([128, ET2, 2, K], f8)
        with tc.tile_pool(name="stage8", bufs=1) as stage8:
            for et in range(ET2):
                wst8 = stage8.tile([128, 2, K], f32, tag="wst8", bufs=2)
                nc.sync.dma_start(
                    wst8[:],
                    w_d[K + et * 256:K + (et + 1) * 256, :]
                    .rearrange("(p j) k -> p j k", j=2))
                nc.vector.tensor_scalar_mul(w8[:, et, :, :], wst8[:], WSCALE)

        # prologue: casts for b0..b2 stream while weights load (b0 split
        # finer so its first-half completion comes sooner)
        for q in range(2 * ST):
            nc.gpsimd.dma_start(
                enc8_d[0, q * 64:(q + 1) * 64, :],
                enc_d[0, q * 64:(q + 1) * 64, :])
        for b in range(1, 3):
            stage_cast(b)
        # b0's transpose split into two separate half tiles so compute can
        # start after only half the casts; b0's compute loop is sb-major
        encT0h = []
        for h in range(SB):
            eh = tp_pool.tile([128, ET2, NB], u16, tag=f"encT0h{h}",
                              name=f"encT0h{h}")
            nc.sync.dma_start(
                eh[:],
                enc8_d[0, h * NB:(h + 1) * NB, :].bitcast(u16),
                transpose=True)
            encT0h.append(eh)
        encTs = {1: stage_tp(1)}

        ones_bf = const.tile([1, BL], bf16)
        nc.vector.memset(ones_bf[:], 1.0)

        # mask prep early (independent of scores)
        mask_i = fin.tile([BL, S], i32)
        nc.sync.dma_start(mask_i[:], mask_d[:])
        mask_f = fin.tile([BL, S], f32)
        nc.vector.tensor_copy(mask_f[:], mask_i[:])
        t2 = fin.tile([BL, S], f32)
        nc.vector.tensor_scalar(t2[:], mask_f[:], -NEG, NEG, Alu.mult, Alu.add)

        hpb = const.tile([128, KT * BL], f32)  # col = kt*BL + b

        def emit_hp():
            # h_proj[k, b] = sum_d Wh[d, k]*hidden[b, d] + b_attn[k]
            hp_ps = pre_ps.tile([128, NB], f32, tag="pre", name="hp_ps")
            for kt in range(KT):
                for dt in range(KT):
                    nc.tensor.matmul(
                        hp_ps[:, kt * BL:(kt + 1) * BL],
                        wh_bf[:, dt, kt * 128:(kt + 1) * 128],
                        hT_bf[:, dt, :],
                        start=(dt == 0),
                        stop=False,
                    )
                nc.tensor.matmul(
                    hp_ps[:, kt * BL:(kt + 1) * BL],
                    b_attn_bf[:, kt * 128:(kt + 1) * 128],
                    ones_bf[:],
                    start=False,
                    stop=True,
                )
            nc.vector.tensor_copy(hpb[:], hp_ps[:, :KT * BL])

        # h_proj before the loop: Wh loads first in the DMA order, so this
        # completes well before the first tanh needs hpb
        emit_hp()

        # scores accumulate on PSUM partition 0; staged flat on SBUF partition 0
        scores_flat = fin.tile([1, BL * S], f32)

        # ---- main loop over local batches (software-pipelined, 2 deep) ----
        for b in range(BL):
            if b + 3 < BL:
                stage_cast(b + 3)
            if b + 2 < BL:
                encTs[b + 2] = stage_tp(b + 2)
            if b > 0:
                encT8 = encTs.pop(b)
                # rhs view: [p, et', j, s]; j = fp8 pair index inside u16
                rhsv = encT8[:].bitcast(f8).rearrange(
                    "p et (s j) -> p et j s", j=2)
            sc = [sc_ps.tile([1, NB], f32, tag=f"sc{i}", name=f"sc{i}")
                  for i in range(SB)]
            if b == 0:
                # sb-major: start on the first transposed s-half immediately
                for sb in range(SB):
                    rh = encT0h[sb][:].bitcast(f8).rearrange(
                        "p et (s j) -> p et j s", j=2)
                    ths = {}
                    for kt in range(KT):
                        pre = pre_ps.tile([128, NB], f32, tag="pre",
                                          name="preh")
                        for et in range(ET2):
                            nc.tensor.matmul(
                                pre[:],
                                w8[:, et, :, kt * 128:(kt + 1) * 128],
                                rh[:, et, :, :],
                                start=(et == 0),
                                stop=(et == ET2 - 1),
                                perf_mode=DR,
                            )
                        th = tanh_pool.tile([128, NB], bf16, tag="thh",
                                            name="thh", bufs=4)
                        nc.scalar.activation(
                            th[:], pre[:], Tanh,
                            bias=hpb[:, kt * BL:kt * BL + 1],
                            scale=1.0 / WSCALE,
                        )
                        ths[kt] = th
                        if kt > 0:
                            nc.tensor.matmul(
                                sc[sb][:], v_bf[:, kt - 1:kt],
                                ths[kt - 1][:],
                                start=(kt - 1 == 0), stop=False)
                    nc.tensor.matmul(
                        sc[sb][:], v_bf[:, KT - 1:KT], ths[KT - 1][:],
                        start=False, stop=True)
                    nc.vector.tensor_copy(
                        scores_flat[:, sb * NB:(sb + 1) * NB], sc[sb][:])
            else:
                ths = {}
                for kt in range(KT):
                    pre = pre_ps.tile([128, SB * NB], f32, tag="pre")
                    for et in range(ET2):  # one LDWEIGHTS serves both sb
                        for sb in range(SB):
                            nc.tensor.matmul(
                                pre[:, sb * NB:(sb + 1) * NB],
                                w8[:, et, :, kt * 128:(kt + 1) * 128],
                                rhsv[:, et, :, sb * NB:(sb + 1) * NB],
                                start=(et == 0),
                                stop=(et == ET2 - 1),
                                perf_mode=DR,
                            )
                    th = tanh_pool.tile([128, SB * NB], bf16, tag="tanh")
                    nc.scalar.activation(
                        th[:], pre[:], Tanh,
                        bias=hpb[:, kt * BL + b:kt * BL + b + 1],
                        scale=1.0 / WSCALE,
                    )
                    ths[kt] = th
                    if kt > 1:
                        for sb in range(SB):
                            nc.tensor.matmul(
                                sc[sb][:], v_bf[:, kt - 2:kt - 1],
                                ths[kt - 2][:, sb * NB:(sb + 1) * NB],
                                start=(kt - 2 == 0), stop=False)
                for kt in (KT - 2, KT - 1):
                    for sb in range(SB):
                        nc.tensor.matmul(
                            sc[sb][:], v_bf[:, kt:kt + 1],
                            ths[kt][:, sb * NB:(sb + 1) * NB],
                            start=False, stop=(kt == KT - 1))
                for sb in range(SB):
                    nc.vector.tensor_copy(
                        scores_flat[:, b * S + sb * NB:b * S + (sb + 1) * NB],
                        sc[sb][:])

        # ---- masked softmax over s (all BL rows at once) ----
        # adding (mask-1)*1e10 alone is enough: exp(score-1e10-max) == 0
        scores = fin.tile([BL, S], f32)
        nc.gpsimd.dma_start(scores[:], scores_flat[:])

        masked = fin.tile([BL, S], f32)
        nc.vector.tensor_add(masked[:], scores[:], t2[:])

        negmax = fin.tile([BL, 1], f32)
        nc.vector.tensor_reduce(negmax[:], masked[:], Ax.X, Alu.max, negate=True)
        expv = fin.tile([BL, S], f32)
        rowsum = fin.tile([BL, 1], f32)
        nc.scalar.activation(expv[:], masked[:], Exp, bias=negmax[:], accum_out=rowsum[:])
        recip = fin.tile([BL, 1], f32)
        nc.vector.reciprocal(recip[:], rowsum[:])
        outf = fin.tile([BL, S], f32)
        nc.vector.tensor_scalar_mul(outf[:], expv[:], recip[:])
        nc.sync.dma_start(out_d[:], outf[:])

    if strip:
        _split_multi_waits(nc, mybir)
    return nc


def _split_multi_waits(nc, mybir):
    """Move extra semaphore waits onto standalone NoOps on the same engine.

    This walrus build encodes at most one sync-wait command per instruction,
    but Tile emits instructions with several (cross-engine RAW + WAR + DMA
    queue ordering). A NoOp carrying one wait, placed immediately before the
    instruction in the same engine's stream, is semantically identical: the
    engine's sequencer blocks on the NoOp's wait before dispatching the real
    instruction.
    """
    n = 0
    for fn in nc.m.functions:
        for blk in fn.blocks:
            insts = blk.instructions
            new = []
            changed = False
            for inst in insts:
                si = inst.sync_info
                if si is not None and si.on_wait and len(si.on_wait) > 1:
                    for w in list(si.on_wait)[:-1]:
                        n += 1
                        new.append(mybir.InstNoOp(
                            name=f"{inst.name}-sw{n}",
                            engine=inst.engine,
                            text_hint="split_wait",
                            bass_nofuse=True,
                            sync_info=mybir.SyncInfo(
                                on_wait=[w], on_update=[]),
                        ))
                    inst.sync_info = mybir.SyncInfo(
                        on_wait=[list(si.on_wait)[-1]],
                        on_update=list(si.on_update or []))
                    changed = True
                new.append(inst)
            if changed:
                blk.instructions = new


def get_nc(strip=True):
    key = ("nc", strip)
    if key not in _CACHE:
        _CACHE[key] = _build_bass(strip)
    return _CACHE[key]


def make_in_maps(hidden, encoder_outputs, mask, W_attn, b_attn, v):
    b_attn = np.ascontiguousarray(np.asarray(b_attn, dtype=np.float32))
    v_pt = np.ascontiguousarray(np.asarray(v, dtype=np.float32).reshape(8, 128).T)
    W_attn = np.ascontiguousarray(np.asarray(W_attn, dtype=np.float32))
    in_maps = []
    for c in range(NCORES):
        sl = slice(c * BL, (c + 1) * BL)
        in_maps.append({
            "encoder_outputs": np.ascontiguousarray(encoder_outputs[sl]),
            "mask": np.ascontiguousarray(np.asarray(mask[sl], dtype=np.int32)),
            "W_attn": W_attn,
            "hiddenT": np.ascontiguousarray(np.asarray(hidden[sl]).T),
            "b_attn": b_attn,
            "v_pt": v_pt,
        })
    return in_maps


def kernel(hidden, encoder_outputs, mask, W_attn, b_attn, v):
    from concourse.bass_utils import run_bass_kernel_spmd

    nc = get_nc()
    in_maps = make_in_maps(hidden, encoder_outputs, mask, W_attn, b_attn, v)
    res = run_bass_kernel_spmd(nc, in_maps, core_ids=list(range(NCORES)))
    return np.concatenate(
        [np.asarray(res.results[c]["out"], dtype=np.float32) for c in range(NCORES)],
        axis=0,
    )

